# revision 1
# baseline (speedup 1.0000x reference)
"""AttentionPooling (segment softmax-pool) Trainium2 kernel.

Graphs are sharded across 8 cores (1024 graphs each); nodes follow their graph
(batch is sorted). Each core's graphs form 8 windows of 128 graphs; a window's
nodes are host-padded to a fixed count T and processed in groups of 512.

out[g] = (sum_{n in g} e_n * x_n) / (sum_{n in g} e_n + 1e-8), with
e_n = exp(tanh(x_n @ W1 + b1) @ W2 + b2); the division is pulled out of the
node loop so one pass over x suffices.

Per 512-node group (all matmuls bf16, fp32 accumulate):
  mm1:  h^T[hid_out, node] = W1_chunk.T @ x^T   (x^T shipped pre-transposed)
  tanh: one ACT op PSUM->SBUF bf16 (bias folded; fused [128,1024] if b1==0)
  mm2:  logits[node,1] = (h^T chunk as stationary).T @ W2_chunk, k-accumulated
  exp:  one ACT op per group on [128,4] logits (+b2)
  S[node, graph] = (iota == batch_rel) * e    (one fused DVE tensor_scalar)
  seg:  psum[graph, 0:257] += S.T @ [x | 1]   (accumulated over the window)
Window end: out = psum[:,0:256] / (psum[:,256] + eps) -> one DMA.

The group stages run as a 2-deep software pipeline across the flattened
(window, group) sequence: each step PE executes seg(i-2), mm1(i), mm2(i-1),
so the ACT tanh/exp latency of a group hides under real PE work. x is shipped
in BOTH layouts (natural rows [x|1|pad] and transposed), host pre-swizzled so
each window is one contiguous [128, ~32KB-per-partition] DMA at line rate,
prefetched one window ahead.
"""
import os
import sys

for _p in ("/opt/trn_rl_repo", "/root/.axon_site/_ro/trn_rl_repo"):
    if os.path.isdir(_p) and _p not in sys.path:
        sys.path.insert(0, _p)

import numpy as np
import ml_dtypes

import concourse.bacc as bacc
import concourse.tile as tile
from concourse import mybir
from concourse.bass_utils import run_bass_kernel_spmd

F32 = mybir.dt.float32
BF16 = mybir.dt.bfloat16
BF = ml_dtypes.bfloat16

N_GRAPHS = 8192
HIDDEN = 256
CORES = 8
WPC = 8            # windows per core
WG = 128           # graphs per window
GRP = 512          # nodes per group
ROW = 258          # xn row: 256 x + 1.0 + pad
EPS = 1e-8

# const block (bf16): W1 blocks (k,m) at 128*(2k+m); W2 chunk k at 512+k;
# iota row at 514:642
C_W1 = 0
C_W2 = 512
C_IOTA = 514
CBW = 642


def _build_program(T: int, reps: int = 1, variant: str = "full",
                   zero_bias: bool = False):
    """variant: "full" | "dma" (loads only)."""
    ng = T // GRP
    cols = T // 128
    XNW = ng * 4 * ROW
    XTW = ng * 1024

    nc = bacc.Bacc("TRN2", target_bir_lowering=False, debug=False,
                   num_devices=CORES)
    xn = nc.dram_tensor("xn", [WPC, 128, XNW], BF16, kind="ExternalInput").ap()
    xt = nc.dram_tensor("xt", [WPC, 128, XTW], BF16, kind="ExternalInput").ap()
    br = nc.dram_tensor("br", [WPC, 128, cols], F32, kind="ExternalInput").ap()
    cbf = nc.dram_tensor("cbf", [128, CBW], BF16, kind="ExternalInput").ap()
    cf = nc.dram_tensor("cf", [128, 3], F32, kind="ExternalInput").ap()
    out = nc.dram_tensor("out", [WPC * WG, HIDDEN], F32, kind="ExternalOutput").ap()

    from contextlib import ExitStack
    with tile.TileContext(nc) as tc:
        with ExitStack() as ctx:
            cpool = ctx.enter_context(tc.tile_pool(name="const", bufs=1))
            brpool = ctx.enter_context(tc.tile_pool(name="brp", bufs=2))
            xnpool = ctx.enter_context(tc.tile_pool(name="xnp", bufs=2))
            xtpool = ctx.enter_context(tc.tile_pool(name="xtp", bufs=2))
            deep = 1 if variant == "fullb" else 0
            htpool = ctx.enter_context(tc.tile_pool(name="htp", bufs=2 + deep))
            etpool = ctx.enter_context(tc.tile_pool(name="etp", bufs=3 + deep))
            spool = ctx.enter_context(tc.tile_pool(name="sp", bufs=12 if variant == "full2" else 8))
            owpool = ctx.enter_context(tc.tile_pool(name="ow", bufs=2))
            phpool = ctx.enter_context(tc.tile_pool(name="ph", bufs=2, space="PSUM"))
            plpool = ctx.enter_context(tc.tile_pool(name="pl", bufs=2, space="PSUM"))
            pgpool = ctx.enter_context(tc.tile_pool(name="pg", bufs=2, space="PSUM"))
            if reps > 1:
                ctx.enter_context(tc.For_i(0, reps, 1))

            cb = cpool.tile([128, CBW], BF16)
            cft = cpool.tile([128, 3], F32)
            nc.sync.dma_start(out=cb[:], in_=cbf[:])
            nc.sync.dma_start(out=cft[:], in_=cf[:])
            iota = cb[:, C_IOTA:C_IOTA + 128]

            wstate = {}

            def load_window(w):
                brw = brpool.tile([128, cols], F32)
                nc.sync.dma_start(out=brw[:], in_=br[w])
                xnwt = xnpool.tile([128, XNW], BF16)
                nc.sync.dma_start(out=xnwt[:], in_=xn[w])
                xtwt = xtpool.tile([128, XTW], BF16)
                nc.sync.dma_start(out=xtwt[:], in_=xt[w])
                pseg = pgpool.tile([128, 257], F32)
                wstate[w] = dict(brw=brw, xnwt=xnwt, xtwt=xtwt, pseg=pseg)

            if variant == "dma":
                for w in range(WPC):
                    load_window(w)
                    ws = wstate[w]
                    dum = etpool.tile([128, 1], F32)
                    nc.vector.tensor_scalar(dum[:], ws["xnwt"][:, 0:1], 1.0,
                                            None, op0=mybir.AluOpType.mult)
                    dum2 = etpool.tile([128, 1], F32)
                    nc.vector.tensor_scalar(dum2[:], ws["xtwt"][:, 0:1], 1.0,
                                            None, op0=mybir.AluOpType.mult)
                    dum3 = etpool.tile([128, 1], F32)
                    nc.vector.tensor_scalar(dum3[:], ws["brw"][:, 0:1], 1.0,
                                            None, op0=mybir.AluOpType.mult)

            if variant == "pe":
                # PE-only probe: same matmul shapes as the real pipeline,
                # constant operands, no DMA/ACT/DVE per group.
                sconst = cb[:, 0:128]
                rh = cb[:, 0:512]
                rx = cb[:, 0:257]
                for w in range(WPC):
                    pseg = pgpool.tile([128, 257], F32)
                    for g in range(ng):
                        ph = phpool.tile([128, 2, GRP], F32)
                        for m in range(2):
                            nc.tensor.matmul(ph[:, m, :],
                                             cb[:, C_W1 + 128 * m:C_W1 + 128 * (m + 1)],
                                             rh, start=True, stop=False)
                            nc.tensor.matmul(ph[:, m, :],
                                             cb[:, C_W1 + 128 * (2 + m):C_W1 + 128 * (3 + m)],
                                             rh, start=False, stop=True)
                        pl = plpool.tile([128, 4], F32)
                        for t in range(4):
                            for k in range(2):
                                nc.tensor.matmul(pl[:, t:t + 1], sconst,
                                                 cb[:, C_W2 + k:C_W2 + k + 1],
                                                 start=(k == 0), stop=(k == 1))
                        for t in range(4):
                            nc.tensor.matmul(pseg[:], sconst, rx,
                                             start=(g == 0 and t == 0),
                                             stop=(g == ng - 1 and t == 3))
                    dct = owpool.tile([128, 1], F32)
                    nc.vector.tensor_scalar(dct[:], pseg[:, 0:1], 1.0, None,
                                            op0=mybir.AluOpType.mult)
                    dph = owpool.tile([128, 1], F32)
                    nc.vector.tensor_scalar(dph[:], pl[:, 0:1], 1.0, None,
                                            op0=mybir.AluOpType.mult)
                    dp2 = owpool.tile([128, 1], F32)
                    nc.vector.tensor_scalar(dp2[:], ph[:, 0, 0:1], 1.0, None,
                                            op0=mybir.AluOpType.mult)

            if variant == "act":
                # ACT-only probe: one seed matmul fills psum, then the same
                # tanh/exp op stream as the real pipeline.
                phx = phpool.tile([128, 2, GRP], F32)
                nc.tensor.matmul(phx[:, 0, :], cb[:, 0:128], cb[:, 0:512],
                                 start=True, stop=True)
                nc.tensor.matmul(phx[:, 1, :], cb[:, 0:128], cb[:, 0:512],
                                 start=True, stop=True)
                for w in range(WPC):
                    for g in range(ng):
                        ht = htpool.tile([128, 2, GRP], BF16)
                        nc.scalar.activation(ht[:, :, :], phx[:, :, :],
                                             mybir.ActivationFunctionType.Tanh,
                                             bias=0.0, scale=1.0)
                        et = etpool.tile([128, 4], F32)
                        nc.scalar.activation(et[:], phx[:, 0, 0:4],
                                             mybir.ActivationFunctionType.Exp,
                                             bias=0.0, scale=1.0)
                        dum = owpool.tile([128, 1], F32)
                        nc.vector.tensor_scalar(dum[:], ht[:, 0, 0:1], 1.0,
                                                None, op0=mybir.AluOpType.mult)
                        dum2 = owpool.tile([128, 1], F32)
                        nc.vector.tensor_scalar(dum2[:], et[:, 0:1], 1.0,
                                                None, op0=mybir.AluOpType.mult)

            if variant == "nop":
                dnp = owpool.tile([128, 1], F32)
                nc.vector.tensor_scalar(dnp[:], cft[:, 0:1], 1.0, None,
                                        op0=mybir.AluOpType.mult)

            steps = [] if variant in ("dma", "pe", "act", "nop") else \
                [(w, g) for w in range(WPC) for g in range(ng)]
            gstate = {}
            pairstate = {}

            def emit_mm1_tanh(i):
                w, g = steps[i]
                ws = wstate[w]
                ph = phpool.tile([128, 2, GRP], F32)
                for m in range(2):
                    nc.tensor.matmul(ph[:, m, :],
                                     cb[:, C_W1 + 128 * m:C_W1 + 128 * (m + 1)],
                                     ws["xtwt"][:, g * 1024:g * 1024 + 512],
                                     start=True, stop=False)
                    nc.tensor.matmul(ph[:, m, :],
                                     cb[:, C_W1 + 128 * (2 + m):C_W1 + 128 * (3 + m)],
                                     ws["xtwt"][:, g * 1024 + 512:g * 1024 + 1024],
                                     start=False, stop=True)
                ht = htpool.tile([128, 2, GRP], BF16)
                if zero_bias:
                    nc.scalar.activation(ht[:, :, :], ph[:, :, :],
                                         mybir.ActivationFunctionType.Tanh,
                                         bias=0.0, scale=1.0)
                else:
                    for m in range(2):
                        nc.scalar.activation(ht[:, m, :], ph[:, m, :],
                                             mybir.ActivationFunctionType.Tanh,
                                             bias=cft[:, m:m + 1], scale=1.0)
                gstate[i] = dict(ht=ht)

            def emit_mm2_exp_s(i):
                w, g = steps[i]
                ws = wstate[w]
                gs = gstate[i]
                ht = gs["ht"]
                if variant == "nomm2":
                    dumh = etpool.tile([128, 1], F32)
                    nc.vector.tensor_scalar(dumh[:], ht[:, 0, 0:1], 1.0, None,
                                            op0=mybir.AluOpType.mult)
                    sts = []
                    for t in range(4):
                        st = spool.tile([128, 128], BF16)
                        nc.vector.tensor_scalar(st[:], iota,
                                                ws["brw"][:, g * 4 + t:g * 4 + t + 1],
                                                ws["brw"][:, 0:1],
                                                op0=mybir.AluOpType.is_equal,
                                                op1=mybir.AluOpType.mult)
                        sts.append(st)
                    gs["sts"] = sts
                    return
                def build_s(j, et_ap, base):
                    wj, gj = steps[j]
                    wsj = wstate[wj]
                    sts = []
                    for t in range(4):
                        st = spool.tile([128, 128], BF16)
                        nc.vector.tensor_scalar(st[:], iota,
                                                wsj["brw"][:, gj * 4 + t:gj * 4 + t + 1],
                                                et_ap[:, base + t:base + t + 1],
                                                op0=mybir.AluOpType.is_equal,
                                                op1=mybir.AluOpType.mult)
                        sts.append(st)
                    gstate[j]["sts"] = sts

                ebias = 0.0 if zero_bias else cft[:, 2:3]
                if variant == "full2":
                    if i % 2 == 0:
                        pl = plpool.tile([128, 8], F32)
                        pairstate[i] = pl
                        lo = 0
                    else:
                        pl = pairstate.pop(i - 1)
                        lo = 4
                    for t in range(4):
                        for k in range(2):
                            nc.tensor.matmul(pl[:, lo + t:lo + t + 1],
                                             ht[:, k, 128 * t:128 * (t + 1)],
                                             cb[:, C_W2 + k:C_W2 + k + 1],
                                             start=(k == 0), stop=(k == 1))
                    if i % 2 == 1:
                        et = etpool.tile([128, 8], F32)
                        nc.scalar.activation(et[:], pl[:],
                                             mybir.ActivationFunctionType.Exp,
                                             bias=ebias, scale=1.0)
                        build_s(i - 1, et, 0)
                        build_s(i, et, 4)
                    return
                pl = plpool.tile([128, 4], F32)
                for t in range(4):
                    for k in range(2):
                        nc.tensor.matmul(pl[:, t:t + 1],
                                         ht[:, k, 128 * t:128 * (t + 1)],
                                         cb[:, C_W2 + k:C_W2 + k + 1],
                                         start=(k == 0), stop=(k == 1))
                et = etpool.tile([128, 4], F32)
                nc.scalar.activation(et[:], pl[:],
                                     mybir.ActivationFunctionType.Exp,
                                     bias=ebias, scale=1.0)
                build_s(i, et, 0)

            def emit_seg(i):
                w, g = steps[i]
                ws = wstate[w]
                gs = gstate.pop(i)
                for t in range(4):
                    nc.tensor.matmul(ws["pseg"][:],
                                     gs["sts"][t][:],
                                     ws["xnwt"][:, (g * 4 + t) * ROW:
                                                (g * 4 + t) * ROW + 257],
                                     start=(g == 0 and t == 0),
                                     stop=(g == ng - 1 and t == 3))
                if g == ng - 1:
                    finalize_window(w)

            def finalize_window(w):
                ws = wstate.pop(w)
                pseg = ws["pseg"]
                dtmp = owpool.tile([128, 1], F32)
                nc.vector.tensor_scalar_add(dtmp[:], pseg[:, 256:257], EPS)
                rec = owpool.tile([128, 1], F32)
                nc.vector.reciprocal(rec[:], dtmp[:])
                ow = owpool.tile([128, HIDDEN], F32)
                nc.vector.tensor_scalar(ow[:], pseg[:, 0:256], rec[:], None,
                                        op0=mybir.AluOpType.mult)
                nc.sync.dma_start(out=out[w * WG:(w + 1) * WG, :], in_=ow[:])

            if steps:
                load_window(0)
            n = len(steps)
            lag = 3 if variant == "full2" else 2
            mm1_first = variant in ("fullr",)
            for i in range(n + lag if n else 0):
                if i < n:
                    w, g = steps[i]
                    if g == lag and w + 1 < WPC:
                        load_window(w + 1)
                if mm1_first:
                    if i < n:
                        emit_mm1_tanh(i)
                    if i >= lag:
                        emit_seg(i - lag)
                else:
                    if i >= lag:
                        emit_seg(i - lag)
                    if i < n:
                        emit_mm1_tanh(i)
                if 1 <= i <= n:
                    emit_mm2_exp_s(i - 1)
    nc.compile()
    return nc


def _prep_inputs(x, batch, W1, b1, W2, b2):
    batch = np.asarray(batch).astype(np.int64)
    x = np.asarray(x, dtype=np.float32)

    bnds = np.searchsorted(batch, np.arange(0, N_GRAPHS + 1, WG))
    sizes = np.diff(bnds)
    T = int(max(512, ((int(sizes.max()) + GRP - 1) // GRP) * GRP))
    ng = T // GRP
    cols = T // 128

    xbf = x.astype(BF)
    batf = batch.astype(np.float32)

    cbf = np.zeros((128, CBW), dtype=BF)
    W1 = np.asarray(W1, np.float32)
    for k in range(2):
        for m in range(2):
            cbf[:, C_W1 + 128 * (2 * k + m):C_W1 + 128 * (2 * k + m + 1)] = \
                W1[128 * k:128 * (k + 1), 128 * m:128 * (m + 1)].astype(BF)
        cbf[:, C_W2 + k] = np.asarray(W2, np.float32)[128 * k:128 * (k + 1), 0].astype(BF)
    cbf[:, C_IOTA:C_IOTA + 128] = np.tile(
        np.arange(128, dtype=np.float32), (128, 1)).astype(BF)

    cf = np.zeros((128, 3), dtype=np.float32)
    cf[:, 0] = np.asarray(b1, np.float32)[0:128]
    cf[:, 1] = np.asarray(b1, np.float32)[128:256]
    cf[:, 2] = float(np.asarray(b2, np.float32).reshape(-1)[0])
    zero_bias = bool((np.asarray(b1) == 0).all() and (np.asarray(b2) == 0).all())

    in_maps = []
    for c in range(CORES):
        xn_c = np.zeros((WPC, T, ROW), dtype=BF)
        xt_lin = np.zeros((WPC, 2, 128, T), dtype=BF)
        br_c = np.full((WPC, 128, cols), -1.0, dtype=np.float32)
        for w in range(WPC):
            j = c * WPC + w
            s, e = int(bnds[j]), int(bnds[j + 1])
            sz = e - s
            if sz:
                xn_c[w, :sz, 0:256] = xbf[s:e]
                xn_c[w, :sz, 256] = BF(1.0)
                xt_lin[w, 0, :, :sz] = xbf[s:e, 0:128].T
                xt_lin[w, 1, :, :sz] = xbf[s:e, 128:256].T
                tmp = np.full(T, -1.0, dtype=np.float32)
                tmp[:sz] = batf[s:e] - (c * 1024 + w * WG)
                br_c[w] = tmp.reshape(cols, 128).T
        # xn swizzle: [w, g*512+t*128+p, d] -> [w, p, (g*4+t)*ROW + d]
        xn_sw = np.ascontiguousarray(
            xn_c.reshape(WPC, ng, 4, 128, ROW).transpose(0, 3, 1, 2, 4)
        ).reshape(WPC, 128, ng * 4 * ROW)
        # xt swizzle: [w, c2, p, g*512+n] -> [w, p, (g*2+c2)*512 + n]
        xt_sw = np.ascontiguousarray(
            xt_lin.reshape(WPC, 2, 128, ng, 512).transpose(0, 2, 3, 1, 4)
        ).reshape(WPC, 128, ng * 1024)
        in_maps.append(dict(xn=xn_sw, xt=xt_sw, br=br_c, cbf=cbf, cf=cf))
    return T, in_maps, zero_bias


_PROGRAM_CACHE = {}


def kernel(x, batch, W1, b1, W2, b2):
    T, in_maps, zb = _prep_inputs(x, batch, W1, b1, W2, b2)
    key = (T, zb)
    if key not in _PROGRAM_CACHE:
        _PROGRAM_CACHE[key] = _build_program(T, zero_bias=zb)
    nc = _PROGRAM_CACHE[key]
    res = run_bass_kernel_spmd(nc, in_maps, list(range(CORES))).results
    return np.concatenate([res[c]["out"] for c in range(CORES)], axis=0)



# revision 12
# speedup vs baseline: 1.0592x; 1.0592x over previous
"""AttentionPooling (segment softmax-pool) Trainium2 kernel — v2.

Graphs are sharded across 8 cores (1024 graphs each); nodes follow their graph
(batch is sorted). Each core's graphs form 8 windows of 128 graphs; a window's
nodes are host-padded to a fixed count T and processed in groups of 512.

out[g] = (sum_{n in g} e_n * x_n) / (sum_{n in g} e_n + 1e-8), with
e_n = exp(tanh(x_n @ W1 + b1) @ W2 + b2); the division is pulled out of the
node loop so one pass over x suffices.

v2 changes vs v1:
  - attention-path x (xt, transposed layout) and W1 ship in fp8e4; mm1 runs
    as 2 DoubleRow matmuls per group (virtual K=256, 2x PE rate, half the
    xt HBM bytes).  Value-path x (xn) stays bf16 (fp8 there fails the 2e-2
    error gate); measured end-to-end rel err ~1.3e-2.
  - exp is batched per window: mm2 logits accumulate into a window-level
    [128, 64] PSUM tile, one Exp ACT op per window (vs 128 tiny ones).
  - window-lag software pipeline: during window w's mm1/tanh/mm2, PE also
    runs window w-1's seg matmuls (S built by DVE from the already-computed
    e's).  PSUM budget: 2x2-bank ph + 2x1-bank pl + 2x1-bank pseg = 8 banks.
  - 3-deep window buffering so DMA prefetch runs 2 windows ahead.

Per 512-node group:
  mm1:  h^T[hid_out, node] = W1_dr.T @ x^T_dr   (fp8 DoubleRow, 2 matmuls)
  tanh: one ACT op PSUM->SBUF bf16 (fused [128,1024] if b1==0)
  mm2:  pl_w[:, g*4+t] += ht_chunk.T @ W2_chunk  (bf16, k-accumulated)
  (window end) exp: one ACT op on [128, 64] logits (+b2)
  S[node, graph] = (iota == batch_rel) * e    (one fused DVE tensor_scalar)
  seg:  psum[graph, 0:257] += S.T @ [x | 1]   (accumulated over the window)
Window end: out = psum[:,0:256] / (psum[:,256] + eps) -> one DMA.
"""
import os
import sys

for _p in ("/opt/trn_rl_repo", "/root/.axon_site/_ro/trn_rl_repo"):
    if os.path.isdir(_p) and _p not in sys.path:
        sys.path.insert(0, _p)

import numpy as np
import ml_dtypes

import concourse.bacc as bacc
import concourse.tile as tile
from concourse import mybir
from concourse.bass_utils import run_bass_kernel_spmd

F32 = mybir.dt.float32
BF16 = mybir.dt.bfloat16
FP8 = mybir.dt.float8e4
BF = ml_dtypes.bfloat16
F8 = ml_dtypes.float8_e4m3

N_GRAPHS = 8192
HIDDEN = 256
CORES = 8
WPC = 8            # windows per core
WG = 128           # graphs per window
GRP = 512          # nodes per group
ROW = 258          # xn row: 256 x + 1.0 + pad
EPS = 1e-8

# bf16 const block: W2 chunk k at col k; iota row at 2:130
C_W2 = 0
C_IOTA = 2
CBW = 130


def _build_program(T: int, variant: str = "full", zero_bias: bool = False):
    ng = T // GRP
    cols = T // 128
    XNW = ng * 4 * ROW
    XTW = ng * 1024

    nc = bacc.Bacc("TRN2", target_bir_lowering=False, debug=False,
                   num_devices=CORES)
    xn = nc.dram_tensor("xn", [WPC, 128, XNW], BF16, kind="ExternalInput").ap()
    xt = nc.dram_tensor("xt", [WPC, 128, ng, 2, GRP], FP8,
                        kind="ExternalInput").ap()
    br = nc.dram_tensor("br", [WPC, 128, cols], F32, kind="ExternalInput").ap()
    cbf = nc.dram_tensor("cbf", [128, CBW], BF16, kind="ExternalInput").ap()
    cw1 = nc.dram_tensor("cw1", [128, 2, 2, 128], FP8,
                         kind="ExternalInput").ap()
    cf = nc.dram_tensor("cf", [128, 3], F32, kind="ExternalInput").ap()
    out = nc.dram_tensor("out", [WPC * WG, HIDDEN], F32, kind="ExternalOutput").ap()

    from contextlib import ExitStack
    with tile.TileContext(nc) as tc:
        with ExitStack() as ctx:
            cpool = ctx.enter_context(tc.tile_pool(name="const", bufs=1))
            brpool = ctx.enter_context(tc.tile_pool(name="brp", bufs=3))
            xnpool = ctx.enter_context(tc.tile_pool(name="xnp", bufs=3))
            xtpool = ctx.enter_context(tc.tile_pool(name="xtp", bufs=3))
            htpool = ctx.enter_context(tc.tile_pool(name="htp", bufs=3))
            etpool = ctx.enter_context(tc.tile_pool(name="etp", bufs=2))
            spool = ctx.enter_context(tc.tile_pool(name="sp", bufs=8))
            owpool = ctx.enter_context(tc.tile_pool(name="ow", bufs=2))
            phpool = ctx.enter_context(tc.tile_pool(name="ph", bufs=2, space="PSUM"))
            plpool = ctx.enter_context(tc.tile_pool(name="pl", bufs=2, space="PSUM"))
            pgpool = ctx.enter_context(tc.tile_pool(name="pg", bufs=2, space="PSUM"))

            cb = cpool.tile([128, CBW], BF16)
            cw = cpool.tile([128, 2, 2, 128], FP8)
            cft = cpool.tile([128, 3], F32)
            nc.sync.dma_start(out=cb[:], in_=cbf[:])
            nc.sync.dma_start(out=cw[:], in_=cw1[:])
            nc.sync.dma_start(out=cft[:], in_=cf[:])
            iota = cb[:, C_IOTA:C_IOTA + 128]

            wstate = {}

            def load_window(w):
                if w >= WPC:
                    return
                brw = brpool.tile([128, cols], F32)
                nc.sync.dma_start(out=brw[:], in_=br[w])
                xnwt = xnpool.tile([128, XNW], BF16)
                nc.sync.dma_start(out=xnwt[:], in_=xn[w])
                xtwt = xtpool.tile([128, ng, 2, GRP], FP8)
                nc.sync.dma_start(out=xtwt[:], in_=xt[w])
                wstate[w] = dict(brw=brw, xnwt=xnwt, xtwt=xtwt)

            def emit_mm1_tanh(w, g):
                ws = wstate[w]
                ph = phpool.tile([128, 2, GRP], F32)
                xt3 = ws["xtwt"][:, g]
                for m in range(2):
                    nc.tensor.matmul(ph[:, m, :], cw[:, m], xt3,
                                     start=True, stop=True,
                                     perf_mode=mybir.MatmulPerfMode.DoubleRow)
                ht = htpool.tile([128, 2, GRP], BF16)
                if zero_bias:
                    nc.scalar.activation(ht[:, :, :], ph[:, :, :],
                                         mybir.ActivationFunctionType.Tanh,
                                         bias=0.0, scale=1.0)
                else:
                    for m in range(2):
                        nc.scalar.activation(ht[:, m, :], ph[:, m, :],
                                             mybir.ActivationFunctionType.Tanh,
                                             bias=cft[:, m:m + 1], scale=1.0)
                ws.setdefault("ht", {})[g] = ht

            def emit_mm2(w, g):
                ws = wstate[w]
                if g == 0:
                    ws["plw"] = plpool.tile([128, cols], F32, name="plw")
                ht = ws["ht"].pop(g)
                plw = ws["plw"]
                for t in range(4):
                    c = g * 4 + t
                    for k in range(2):
                        nc.tensor.matmul(plw[:, c:c + 1],
                                         ht[:, k, 128 * t:128 * (t + 1)],
                                         cb[:, C_W2 + k:C_W2 + k + 1],
                                         start=(k == 0), stop=(k == 1))

            def emit_exp(w):
                ws = wstate[w]
                plw = ws.pop("plw")
                et = etpool.tile([128, cols], F32)
                ebias = 0.0 if zero_bias else cft[:, 2:3]
                nc.scalar.activation(et[:], plw[:],
                                     mybir.ActivationFunctionType.Exp,
                                     bias=ebias, scale=1.0)
                ws["et"] = et
                ws["pseg"] = pgpool.tile([128, 257], F32, name="pseg")

            def emit_sbuild_seg(w, g):
                ws = wstate[w]
                sts = []
                for t in range(4):
                    c = g * 4 + t
                    st = spool.tile([128, 128], BF16)
                    nc.vector.tensor_scalar(st[:], iota,
                                            ws["brw"][:, c:c + 1],
                                            ws["et"][:, c:c + 1],
                                            op0=mybir.AluOpType.is_equal,
                                            op1=mybir.AluOpType.mult)
                    sts.append(st)
                for t in range(4):
                    c = g * 4 + t
                    nc.tensor.matmul(ws["pseg"][:],
                                     sts[t][:],
                                     ws["xnwt"][:, c * ROW:c * ROW + 257],
                                     start=(g == 0 and t == 0),
                                     stop=(g == ng - 1 and t == 3))
                if g == ng - 1:
                    finalize_window(w)

            def finalize_window(w):
                ws = wstate.pop(w)
                pseg = ws["pseg"]
                dtmp = owpool.tile([128, 1], F32)
                nc.vector.tensor_scalar_add(dtmp[:], pseg[:, 256:257], EPS)
                rec = owpool.tile([128, 1], F32)
                nc.vector.reciprocal(rec[:], dtmp[:])
                ow = owpool.tile([128, HIDDEN], F32)
                nc.vector.tensor_scalar(ow[:], pseg[:, 0:256], rec[:], None,
                                        op0=mybir.AluOpType.mult)
                nc.sync.dma_start(out=out[w * WG:(w + 1) * WG, :], in_=ow[:])

            # main pipeline: iteration w runs pass1 of window w and pass2
            # (seg) of window w-1
            load_window(0)
            load_window(1)
            for w in range(WPC + 1):
                for g in range(ng):
                    if w >= 1:
                        emit_sbuild_seg(w - 1, g)
                    if w < WPC:
                        emit_mm1_tanh(w, g)
                        emit_mm2(w, g)
                    if g == 2:
                        load_window(w + 2)
                if w < WPC:
                    emit_exp(w)
    nc.compile()
    return nc


def _prep_inputs(x, batch, W1, b1, W2, b2):
    batch = np.asarray(batch).astype(np.int64)
    x = np.asarray(x, dtype=np.float32)

    bnds = np.searchsorted(batch, np.arange(0, N_GRAPHS + 1, WG))
    sizes = np.diff(bnds)
    T = int(max(512, ((int(sizes.max()) + GRP - 1) // GRP) * GRP))
    ng = T // GRP
    cols = T // 128

    xbf = x.astype(BF)
    x8 = x.astype(F8)
    batf = batch.astype(np.float32)

    W1 = np.asarray(W1, np.float32)
    cbf = np.zeros((128, CBW), dtype=BF)
    for k in range(2):
        cbf[:, C_W2 + k] = np.asarray(W2, np.float32)[128 * k:128 * (k + 1), 0].astype(BF)
    cbf[:, C_IOTA:C_IOTA + 128] = np.tile(
        np.arange(128, dtype=np.float32), (128, 1)).astype(BF)

    # W1 DoubleRow: cw1[p, mb, i, m] = W1[p + 128*i, 128*mb + m]
    cw1 = np.zeros((128, 2, 2, 128), dtype=F8)
    for mb in range(2):
        for i in range(2):
            cw1[:, mb, i, :] = \
                W1[128 * i:128 * (i + 1), 128 * mb:128 * (mb + 1)].astype(F8)

    cf = np.zeros((128, 3), dtype=np.float32)
    cf[:, 0] = np.asarray(b1, np.float32)[0:128]
    cf[:, 1] = np.asarray(b1, np.float32)[128:256]
    cf[:, 2] = float(np.asarray(b2, np.float32).reshape(-1)[0])
    zero_bias = bool((np.asarray(b1) == 0).all() and (np.asarray(b2) == 0).all())

    in_maps = []
    for c in range(CORES):
        xn_c = np.zeros((WPC, T, ROW), dtype=BF)
        xt_lin = np.zeros((WPC, 2, 128, T), dtype=F8)
        br_c = np.full((WPC, 128, cols), -1.0, dtype=np.float32)
        for w in range(WPC):
            j = c * WPC + w
            s, e = int(bnds[j]), int(bnds[j + 1])
            sz = e - s
            if sz:
                xn_c[w, :sz, 0:256] = xbf[s:e]
                xn_c[w, :sz, 256] = BF(1.0)
                xt_lin[w, 0, :, :sz] = x8[s:e, 0:128].T
                xt_lin[w, 1, :, :sz] = x8[s:e, 128:256].T
                tmp = np.full(T, -1.0, dtype=np.float32)
                tmp[:sz] = batf[s:e] - (c * 1024 + w * WG)
                br_c[w] = tmp.reshape(cols, 128).T
        # xn swizzle: [w, g*512+t*128+p, d] -> [w, p, (g*4+t)*ROW + d]
        xn_sw = np.ascontiguousarray(
            xn_c.reshape(WPC, ng, 4, 128, ROW).transpose(0, 3, 1, 2, 4)
        ).reshape(WPC, 128, ng * 4 * ROW)
        # xt swizzle: [w, c2, p, g*512+n] -> [w, p, g, c2, n]
        xt_sw = np.ascontiguousarray(
            xt_lin.reshape(WPC, 2, 128, ng, 512).transpose(0, 2, 3, 1, 4))
        in_maps.append(dict(xn=xn_sw, xt=xt_sw, br=br_c, cbf=cbf, cw1=cw1, cf=cf))
    return T, in_maps, zero_bias


_PROGRAM_CACHE = {}


def kernel(x, batch, W1, b1, W2, b2):
    T, in_maps, zb = _prep_inputs(x, batch, W1, b1, W2, b2)
    key = (T, zb)
    if key not in _PROGRAM_CACHE:
        _PROGRAM_CACHE[key] = _build_program(T, zero_bias=zb)
    nc = _PROGRAM_CACHE[key]
    res = run_bass_kernel_spmd(nc, in_maps, list(range(CORES))).results
    return np.concatenate([res[c]["out"] for c in range(CORES)], axis=0)


# revision 14
# speedup vs baseline: 1.2436x; 1.1740x over previous
"""AttentionPooling (segment softmax-pool) Trainium2 kernel — v2.

Graphs are sharded across 8 cores (1024 graphs each); nodes follow their graph
(batch is sorted). Each core's graphs form 8 windows of 128 graphs; a window's
nodes are host-padded to a fixed count T and processed in groups of 512.

out[g] = (sum_{n in g} e_n * x_n) / (sum_{n in g} e_n + 1e-8), with
e_n = exp(tanh(x_n @ W1 + b1) @ W2 + b2); the division is pulled out of the
node loop so one pass over x suffices.

v2 changes vs v1:
  - attention-path x (xt, transposed layout) and W1 ship in fp8e4; mm1 runs
    as 2 DoubleRow matmuls per group (virtual K=256, 2x PE rate, half the
    xt HBM bytes).  Value-path x (xn) stays bf16 (fp8 there fails the 2e-2
    error gate); measured end-to-end rel err ~1.3e-2.
  - exp is batched per window: mm2 logits accumulate into a window-level
    [128, 64] PSUM tile, one Exp ACT op per window (vs 128 tiny ones).
  - window-lag software pipeline: during window w's mm1/tanh/mm2, PE also
    runs window w-1's seg matmuls (S built by DVE from the already-computed
    e's).  PSUM budget: 2x2-bank ph + 2x1-bank pl + 2x1-bank pseg = 8 banks.
  - 3-deep window buffering so DMA prefetch runs 2 windows ahead.

Per 512-node group:
  mm1:  h^T[hid_out, node] = W1_dr.T @ x^T_dr   (fp8 DoubleRow, 2 matmuls)
  tanh: one ACT op PSUM->SBUF bf16 (fused [128,1024] if b1==0)
  mm2:  pl_w[:, g*4+t] += ht_chunk.T @ W2_chunk  (bf16, k-accumulated)
  (window end) exp: one ACT op on [128, 64] logits (+b2)
  S[node, graph] = (iota == batch_rel) * e    (one fused DVE tensor_scalar)
  seg:  psum[graph, 0:257] += S.T @ [x | 1]   (accumulated over the window)
Window end: out = psum[:,0:256] / (psum[:,256] + eps) -> one DMA.
"""
import os
import sys

for _p in ("/opt/trn_rl_repo", "/root/.axon_site/_ro/trn_rl_repo"):
    if os.path.isdir(_p) and _p not in sys.path:
        sys.path.insert(0, _p)

import numpy as np
import ml_dtypes

import concourse.bacc as bacc
import concourse.tile as tile
from concourse import mybir
from concourse.bass_utils import run_bass_kernel_spmd

F32 = mybir.dt.float32
BF16 = mybir.dt.bfloat16
FP8 = mybir.dt.float8e4
BF = ml_dtypes.bfloat16
F8 = ml_dtypes.float8_e4m3

N_GRAPHS = 8192
HIDDEN = 256
CORES = 8
WPC = 8            # windows per core
WG = 128           # graphs per window
GRP = 512          # nodes per group
ROW = 258          # xn row: 256 x + 1.0 + pad
EPS = 1e-8

# bf16 const block: W2 chunk k at col k; iota row at 2:130
C_W2 = 0
C_IOTA = 2
CBW = 130


def _build_program(T: int, variant: str = "full", zero_bias: bool = False):
    ng = T // GRP
    cols = T // 128
    XNW = ng * 4 * ROW
    XTW = ng * 1024

    nc = bacc.Bacc("TRN2", target_bir_lowering=False, debug=False,
                   num_devices=CORES)
    xn = nc.dram_tensor("xn", [WPC, 128, XNW], BF16, kind="ExternalInput").ap()
    xt = nc.dram_tensor("xt", [WPC, 128, ng, 2, GRP], FP8,
                        kind="ExternalInput").ap()
    br = nc.dram_tensor("br", [WPC, 128, cols], F32, kind="ExternalInput").ap()
    cbf = nc.dram_tensor("cbf", [128, CBW], BF16, kind="ExternalInput").ap()
    cw1 = nc.dram_tensor("cw1", [128, 2, 2, 128], FP8,
                         kind="ExternalInput").ap()
    cf = nc.dram_tensor("cf", [128, 3], F32, kind="ExternalInput").ap()
    out = nc.dram_tensor("out", [WPC * WG, HIDDEN], F32, kind="ExternalOutput").ap()

    from contextlib import ExitStack
    with tile.TileContext(nc) as tc:
        with ExitStack() as ctx:
            cpool = ctx.enter_context(tc.tile_pool(name="const", bufs=1))
            brpool = ctx.enter_context(tc.tile_pool(name="brp", bufs=3))
            xnpool = ctx.enter_context(tc.tile_pool(name="xnp", bufs=3))
            xtpool = ctx.enter_context(tc.tile_pool(name="xtp", bufs=3))
            htpool = ctx.enter_context(tc.tile_pool(name="htp", bufs=3))
            etpool = ctx.enter_context(tc.tile_pool(name="etp", bufs=2))
            spool = ctx.enter_context(tc.tile_pool(name="sp", bufs=8))
            owpool = ctx.enter_context(tc.tile_pool(name="ow", bufs=2))
            phpool = ctx.enter_context(tc.tile_pool(name="ph", bufs=2, space="PSUM"))
            plpool = ctx.enter_context(tc.tile_pool(name="pl", bufs=2, space="PSUM"))
            pgpool = ctx.enter_context(tc.tile_pool(name="pg", bufs=2, space="PSUM"))

            cb = cpool.tile([128, CBW], BF16)
            cw = cpool.tile([128, 2, 2, 128], FP8)
            cft = cpool.tile([128, 3], F32)
            nc.sync.dma_start(out=cb[:], in_=cbf[:])
            nc.sync.dma_start(out=cw[:], in_=cw1[:])
            nc.sync.dma_start(out=cft[:], in_=cf[:])
            iota = cb[:, C_IOTA:C_IOTA + 128]

            wstate = {}

            def load_pass1(w):
                # br + xt (chunked): feeds mm1/mm2 of window w
                if w >= WPC:
                    return
                brw = brpool.tile([128, cols], F32)
                nc.sync.dma_start(out=brw[:], in_=br[w])
                xtwt = xtpool.tile([128, ng, 2, GRP], FP8)
                qn = max(1, ng // 4)
                for q in range(0, ng, qn):
                    nc.sync.dma_start(out=xtwt[:, q:q + qn],
                                      in_=xt[w, :, q:q + qn])
                wstate[w] = dict(brw=brw, xtwt=xtwt)

            def load_pass2(w):
                # xn (chunked): feeds seg of window w (runs during iter w+1)
                if w >= WPC:
                    return
                xnwt = xnpool.tile([128, XNW], BF16)
                cq = XNW // 4
                for q in range(4):
                    nc.sync.dma_start(out=xnwt[:, q * cq:(q + 1) * cq],
                                      in_=xn[w, :, q * cq:(q + 1) * cq])
                wstate[w]["xnwt"] = xnwt

            def emit_mm1_tanh(w, g):
                ws = wstate[w]
                ph = phpool.tile([128, 2, GRP], F32)
                xt3 = ws["xtwt"][:, g]
                for m in range(2):
                    nc.tensor.matmul(ph[:, m, :], cw[:, m], xt3,
                                     start=True, stop=True,
                                     perf_mode=mybir.MatmulPerfMode.DoubleRow)
                ht = htpool.tile([128, 2, GRP], BF16)
                if zero_bias:
                    nc.scalar.activation(ht[:, :, :], ph[:, :, :],
                                         mybir.ActivationFunctionType.Tanh,
                                         bias=0.0, scale=1.0)
                else:
                    for m in range(2):
                        nc.scalar.activation(ht[:, m, :], ph[:, m, :],
                                             mybir.ActivationFunctionType.Tanh,
                                             bias=cft[:, m:m + 1], scale=1.0)
                ws.setdefault("ht", {})[g] = ht

            def emit_mm2(w, g):
                ws = wstate[w]
                if g == 0:
                    ws["plw"] = plpool.tile([128, cols], F32, name="plw")
                ht = ws["ht"].pop(g)
                plw = ws["plw"]
                for t in range(4):
                    c = g * 4 + t
                    for k in range(2):
                        nc.tensor.matmul(plw[:, c:c + 1],
                                         ht[:, k, 128 * t:128 * (t + 1)],
                                         cb[:, C_W2 + k:C_W2 + k + 1],
                                         start=(k == 0), stop=(k == 1))

            def emit_exp(w):
                ws = wstate[w]
                plw = ws.pop("plw")
                et = etpool.tile([128, cols], F32)
                ebias = 0.0 if zero_bias else cft[:, 2:3]
                nc.scalar.activation(et[:], plw[:],
                                     mybir.ActivationFunctionType.Exp,
                                     bias=ebias, scale=1.0)
                ws["et"] = et
                ws["pseg"] = pgpool.tile([128, 257], F32, name="pseg")

            def emit_sbuild_seg(w, g):
                ws = wstate[w]
                sts = []
                for t in range(4):
                    c = g * 4 + t
                    st = spool.tile([128, 128], BF16)
                    nc.vector.tensor_scalar(st[:], iota,
                                            ws["brw"][:, c:c + 1],
                                            ws["et"][:, c:c + 1],
                                            op0=mybir.AluOpType.is_equal,
                                            op1=mybir.AluOpType.mult)
                    sts.append(st)
                for t in range(4):
                    c = g * 4 + t
                    nc.tensor.matmul(ws["pseg"][:],
                                     sts[t][:],
                                     ws["xnwt"][:, c * ROW:c * ROW + 257],
                                     start=(g == 0 and t == 0),
                                     stop=(g == ng - 1 and t == 3))
                if g == ng - 1:
                    finalize_window(w)

            def finalize_window(w):
                ws = wstate.pop(w)
                pseg = ws["pseg"]
                dtmp = owpool.tile([128, 1], F32)
                nc.vector.tensor_scalar_add(dtmp[:], pseg[:, 256:257], EPS)
                rec = owpool.tile([128, 1], F32)
                nc.vector.reciprocal(rec[:], dtmp[:])
                ow = owpool.tile([128, HIDDEN], F32)
                nc.vector.tensor_scalar(ow[:], pseg[:, 0:256], rec[:], None,
                                        op0=mybir.AluOpType.mult)
                nc.sync.dma_start(out=out[w * WG:(w + 1) * WG, :], in_=ow[:])

            # main pipeline: iteration w runs pass1 of window w and pass2
            # (seg) of window w-1.  mm1/mm2 are emitted BEFORE seg within
            # each step so the PE never stalls behind the exp->S-build chain
            # at window boundaries.
            load_pass1(0)
            load_pass1(1)
            load_pass2(0)
            for w in range(WPC + 1):
                for g in range(ng):
                    if w < WPC:
                        emit_mm1_tanh(w, g)
                        emit_mm2(w, g)
                    if w >= 1:
                        emit_sbuild_seg(w - 1, g)
                    if g == 2:
                        load_pass1(w + 2)
                    elif g == 6:
                        load_pass2(w + 1)
                if w < WPC:
                    emit_exp(w)
    nc.compile()
    return nc


def _prep_inputs(x, batch, W1, b1, W2, b2):
    batch = np.asarray(batch).astype(np.int64)
    x = np.asarray(x, dtype=np.float32)

    bnds = np.searchsorted(batch, np.arange(0, N_GRAPHS + 1, WG))
    sizes = np.diff(bnds)
    T = int(max(512, ((int(sizes.max()) + GRP - 1) // GRP) * GRP))
    ng = T // GRP
    cols = T // 128

    xbf = x.astype(BF)
    x8 = x.astype(F8)
    batf = batch.astype(np.float32)

    W1 = np.asarray(W1, np.float32)
    cbf = np.zeros((128, CBW), dtype=BF)
    for k in range(2):
        cbf[:, C_W2 + k] = np.asarray(W2, np.float32)[128 * k:128 * (k + 1), 0].astype(BF)
    cbf[:, C_IOTA:C_IOTA + 128] = np.tile(
        np.arange(128, dtype=np.float32), (128, 1)).astype(BF)

    # W1 DoubleRow: cw1[p, mb, i, m] = W1[p + 128*i, 128*mb + m]
    cw1 = np.zeros((128, 2, 2, 128), dtype=F8)
    for mb in range(2):
        for i in range(2):
            cw1[:, mb, i, :] = \
                W1[128 * i:128 * (i + 1), 128 * mb:128 * (mb + 1)].astype(F8)

    cf = np.zeros((128, 3), dtype=np.float32)
    cf[:, 0] = np.asarray(b1, np.float32)[0:128]
    cf[:, 1] = np.asarray(b1, np.float32)[128:256]
    cf[:, 2] = float(np.asarray(b2, np.float32).reshape(-1)[0])
    zero_bias = bool((np.asarray(b1) == 0).all() and (np.asarray(b2) == 0).all())

    in_maps = []
    for c in range(CORES):
        xn_c = np.zeros((WPC, T, ROW), dtype=BF)
        xt_lin = np.zeros((WPC, 2, 128, T), dtype=F8)
        br_c = np.full((WPC, 128, cols), -1.0, dtype=np.float32)
        for w in range(WPC):
            j = c * WPC + w
            s, e = int(bnds[j]), int(bnds[j + 1])
            sz = e - s
            if sz:
                xn_c[w, :sz, 0:256] = xbf[s:e]
                xn_c[w, :sz, 256] = BF(1.0)
                xt_lin[w, 0, :, :sz] = x8[s:e, 0:128].T
                xt_lin[w, 1, :, :sz] = x8[s:e, 128:256].T
                tmp = np.full(T, -1.0, dtype=np.float32)
                tmp[:sz] = batf[s:e] - (c * 1024 + w * WG)
                br_c[w] = tmp.reshape(cols, 128).T
        # xn swizzle: [w, g*512+t*128+p, d] -> [w, p, (g*4+t)*ROW + d]
        xn_sw = np.ascontiguousarray(
            xn_c.reshape(WPC, ng, 4, 128, ROW).transpose(0, 3, 1, 2, 4)
        ).reshape(WPC, 128, ng * 4 * ROW)
        # xt swizzle: [w, c2, p, g*512+n] -> [w, p, g, c2, n]
        xt_sw = np.ascontiguousarray(
            xt_lin.reshape(WPC, 2, 128, ng, 512).transpose(0, 2, 3, 1, 4))
        in_maps.append(dict(xn=xn_sw, xt=xt_sw, br=br_c, cbf=cbf, cw1=cw1, cf=cf))
    return T, in_maps, zero_bias


_PROGRAM_CACHE = {}


def kernel(x, batch, W1, b1, W2, b2):
    T, in_maps, zb = _prep_inputs(x, batch, W1, b1, W2, b2)
    key = (T, zb)
    if key not in _PROGRAM_CACHE:
        _PROGRAM_CACHE[key] = _build_program(T, zero_bias=zb)
    nc = _PROGRAM_CACHE[key]
    res = run_bass_kernel_spmd(nc, in_maps, list(range(CORES))).results
    return np.concatenate([res[c]["out"] for c in range(CORES)], axis=0)


# revision 18
# speedup vs baseline: 1.2490x; 1.0044x over previous
"""AttentionPooling (segment softmax-pool) Trainium2 kernel — v2.

Graphs are sharded across 8 cores (1024 graphs each); nodes follow their graph
(batch is sorted). Each core's graphs form 8 windows of 128 graphs; a window's
nodes are host-padded to a fixed count T and processed in groups of 512.

out[g] = (sum_{n in g} e_n * x_n) / (sum_{n in g} e_n + 1e-8), with
e_n = exp(tanh(x_n @ W1 + b1) @ W2 + b2); the division is pulled out of the
node loop so one pass over x suffices.

v2 changes vs v1:
  - attention-path x (xt, transposed layout) and W1 ship in fp8e4; mm1 runs
    as 2 DoubleRow matmuls per group (virtual K=256, 2x PE rate, half the
    xt HBM bytes).  Value-path x (xn) stays bf16 (fp8 there fails the 2e-2
    error gate); measured end-to-end rel err ~1.3e-2.
  - exp is batched per window: mm2 logits accumulate into a window-level
    [128, 64] PSUM tile, one Exp ACT op per window (vs 128 tiny ones).
  - window-lag software pipeline: during window w's mm1/tanh/mm2, PE also
    runs window w-1's seg matmuls (S built by DVE from the already-computed
    e's).  PSUM budget: 2x2-bank ph + 2x1-bank pl + 2x1-bank pseg = 8 banks.
  - 3-deep window buffering so DMA prefetch runs 2 windows ahead.

Per 512-node group:
  mm1:  h^T[hid_out, node] = W1_dr.T @ x^T_dr   (fp8 DoubleRow, 2 matmuls)
  tanh: one ACT op PSUM->SBUF bf16 (fused [128,1024] if b1==0)
  mm2:  pl_w[:, g*4+t] += ht_chunk.T @ W2_chunk  (bf16, k-accumulated)
  (window end) exp: one ACT op on [128, 64] logits (+b2)
  S[node, graph] = (iota == batch_rel) * e    (one fused DVE tensor_scalar)
  seg:  psum[graph, 0:257] += S.T @ [x | 1]   (accumulated over the window)
Window end: out = psum[:,0:256] / (psum[:,256] + eps) -> one DMA.
"""
import os
import sys

for _p in ("/opt/trn_rl_repo", "/root/.axon_site/_ro/trn_rl_repo"):
    if os.path.isdir(_p) and _p not in sys.path:
        sys.path.insert(0, _p)

import numpy as np
import ml_dtypes

import concourse.bacc as bacc
import concourse.tile as tile
from concourse import mybir
from concourse.bass_utils import run_bass_kernel_spmd

F32 = mybir.dt.float32
BF16 = mybir.dt.bfloat16
FP8 = mybir.dt.float8e4
BF = ml_dtypes.bfloat16
F8 = ml_dtypes.float8_e4m3

N_GRAPHS = 8192
HIDDEN = 256
CORES = 8
WPC = 8            # windows per core
WG = 128           # graphs per window
GRP = 512          # nodes per group
ROW = 258          # xn row: 256 x + 1.0 + pad
EPS = 1e-8

# bf16 const block: W2 chunk k at col k; iota row at 2:130
C_W2 = 0
C_IOTA = 2
CBW = 130


def _build_program(T: int, variant: str = "full", zero_bias: bool = False):
    ng = T // GRP
    cols = T // 128
    XNW = ng * 4 * ROW
    XTW = ng * 1024

    nc = bacc.Bacc("TRN2", target_bir_lowering=False, debug=False,
                   num_devices=CORES)
    xn = nc.dram_tensor("xn", [WPC, 128, XNW], BF16, kind="ExternalInput").ap()
    xt = nc.dram_tensor("xt", [WPC, 128, ng, 2, GRP], FP8,
                        kind="ExternalInput").ap()
    br = nc.dram_tensor("br", [WPC, 128, cols], F32, kind="ExternalInput").ap()
    cbf = nc.dram_tensor("cbf", [128, CBW], BF16, kind="ExternalInput").ap()
    cw1 = nc.dram_tensor("cw1", [128, 2, 2, 128], FP8,
                         kind="ExternalInput").ap()
    cf = nc.dram_tensor("cf", [128, 3], F32, kind="ExternalInput").ap()
    out = nc.dram_tensor("out", [WPC * WG, HIDDEN], F32, kind="ExternalOutput").ap()

    from contextlib import ExitStack
    with tile.TileContext(nc) as tc:
        with ExitStack() as ctx:
            cpool = ctx.enter_context(tc.tile_pool(name="const", bufs=1))
            brpool = ctx.enter_context(tc.tile_pool(name="brp", bufs=3))
            xnpool = ctx.enter_context(tc.tile_pool(name="xnp", bufs=3))
            xtpool = ctx.enter_context(tc.tile_pool(name="xtp", bufs=3))
            htpool = ctx.enter_context(tc.tile_pool(name="htp", bufs=4))
            etpool = ctx.enter_context(tc.tile_pool(name="etp", bufs=3))
            spool = ctx.enter_context(tc.tile_pool(name="sp", bufs=16))
            owpool = ctx.enter_context(tc.tile_pool(name="ow", bufs=3))
            phpool = ctx.enter_context(tc.tile_pool(name="ph", bufs=2, space="PSUM"))
            plpool = ctx.enter_context(tc.tile_pool(name="pl", bufs=2, space="PSUM"))
            pgpool = ctx.enter_context(tc.tile_pool(name="pg", bufs=2, space="PSUM"))

            cb = cpool.tile([128, CBW], BF16)
            cw = cpool.tile([128, 2, 2, 128], FP8)
            cft = cpool.tile([128, 3], F32)
            nc.sync.dma_start(out=cb[:], in_=cbf[:])
            nc.sync.dma_start(out=cw[:], in_=cw1[:])
            nc.sync.dma_start(out=cft[:], in_=cf[:])
            iota = cb[:, C_IOTA:C_IOTA + 128]

            wstate = {}

            def load_pass1(w):
                # br + xt (chunked): feeds mm1/mm2 of window w
                if w >= WPC:
                    return
                brw = brpool.tile([128, cols], F32)
                nc.sync.dma_start(out=brw[:], in_=br[w])
                xtwt = xtpool.tile([128, ng, 2, GRP], FP8)
                qn = max(1, ng // 4)
                for q in range(0, ng, qn):
                    nc.sync.dma_start(out=xtwt[:, q:q + qn],
                                      in_=xt[w, :, q:q + qn])
                wstate[w] = dict(brw=brw, xtwt=xtwt)

            def load_pass2(w):
                # xn (chunked): feeds seg of window w (runs during iter w+1)
                if w >= WPC:
                    return
                xnwt = xnpool.tile([128, XNW], BF16)
                cq = XNW // 4
                for q in range(4):
                    nc.sync.dma_start(out=xnwt[:, q * cq:(q + 1) * cq],
                                      in_=xn[w, :, q * cq:(q + 1) * cq])
                wstate[w]["xnwt"] = xnwt

            def emit_mm1_tanh(w, g):
                ws = wstate[w]
                ph = phpool.tile([128, 2, GRP], F32)
                xt3 = ws["xtwt"][:, g]
                for m in range(2):
                    nc.tensor.matmul(ph[:, m, :], cw[:, m], xt3,
                                     start=True, stop=True,
                                     perf_mode=mybir.MatmulPerfMode.DoubleRow)
                ht = htpool.tile([128, 2, GRP], BF16)
                if zero_bias:
                    nc.scalar.activation(ht[:, :, :], ph[:, :, :],
                                         mybir.ActivationFunctionType.Tanh,
                                         bias=0.0, scale=1.0)
                else:
                    for m in range(2):
                        nc.scalar.activation(ht[:, m, :], ph[:, m, :],
                                             mybir.ActivationFunctionType.Tanh,
                                             bias=cft[:, m:m + 1], scale=1.0)
                ws.setdefault("ht", {})[g] = ht

            def emit_mm2(w, g):
                ws = wstate[w]
                if g == 0:
                    ws["plw"] = plpool.tile([128, cols], F32, name="plw")
                ht = ws["ht"].pop(g)
                plw = ws["plw"]
                for t in range(4):
                    c = g * 4 + t
                    for k in range(2):
                        nc.tensor.matmul(plw[:, c:c + 1],
                                         ht[:, k, 128 * t:128 * (t + 1)],
                                         cb[:, C_W2 + k:C_W2 + k + 1],
                                         start=(k == 0), stop=(k == 1))

            def emit_exp(w, h):
                # exp on half-window h's logit columns
                ws = wstate[w]
                hc = cols // 2
                et = etpool.tile([128, hc], F32)
                ebias = 0.0 if zero_bias else cft[:, 2:3]
                nc.scalar.activation(et[:], ws["plw"][:, h * hc:(h + 1) * hc],
                                     mybir.ActivationFunctionType.Exp,
                                     bias=ebias, scale=1.0)
                ws.setdefault("et", {})[h] = et
                if h == 0:
                    ws["pseg"] = pgpool.tile([128, 257], F32, name="pseg")
                else:
                    ws.pop("plw")

            def emit_sbuild_seg(w, g):
                ws = wstate[w]
                hc = cols // 2
                h = (g * 4) // hc
                et = ws["et"][h]
                sts = []
                for t in range(4):
                    c = g * 4 + t
                    st = spool.tile([128, 128], BF16)
                    nc.vector.tensor_scalar(st[:], iota,
                                            ws["brw"][:, c:c + 1],
                                            et[:, c - h * hc:c - h * hc + 1],
                                            op0=mybir.AluOpType.is_equal,
                                            op1=mybir.AluOpType.mult)
                    sts.append(st)
                for t in range(4):
                    c = g * 4 + t
                    nc.tensor.matmul(ws["pseg"][:],
                                     sts[t][:],
                                     ws["xnwt"][:, c * ROW:c * ROW + 257],
                                     start=(g == 0 and t == 0),
                                     stop=(g == ng - 1 and t == 3))
                if g == ng - 1:
                    finalize_window(w)

            def finalize_window(w):
                ws = wstate.pop(w)
                pseg = ws["pseg"]
                dtmp = owpool.tile([128, 1], F32)
                nc.vector.tensor_scalar_add(dtmp[:], pseg[:, 256:257], EPS)
                rec = owpool.tile([128, 1], F32)
                nc.vector.reciprocal(rec[:], dtmp[:])
                ow = owpool.tile([128, HIDDEN], F32)
                nc.vector.tensor_scalar(ow[:], pseg[:, 0:256], rec[:], None,
                                        op0=mybir.AluOpType.mult)
                nc.sync.dma_start(out=out[w * WG:(w + 1) * WG, :], in_=ow[:])

            # ACT warmup: force the exp/tanh table load at t=0 so the first
            # real tanh doesn't pay the ~2.7us ACT_TABLE_LOAD.
            wz = owpool.tile([128, 1], F32)
            nc.vector.memset(wz[:], 0.0)
            wu = owpool.tile([128, 1], F32)
            nc.scalar.activation(wu[:], wz[:],
                                 mybir.ActivationFunctionType.Exp,
                                 bias=0.0, scale=1.0)

            # main pipeline over flat (window, group) steps; seg lags
            # mm1/mm2 by half a window (exp runs per half-window), so the
            # PE interleaves seg of the previous half with mm1/mm2 of the
            # current one and never stalls behind the exp->S-build chain.
            steps = [(w, g) for w in range(WPC) for g in range(ng)]
            LAG = ng // 2
            n = len(steps)
            load_pass1(0)
            load_pass1(1)
            load_pass2(0)
            for i in range(n + LAG):
                if i < n:
                    w, g = steps[i]
                    emit_mm1_tanh(w, g)
                    emit_mm2(w, g)
                    if g == ng // 2 - 1:
                        emit_exp(w, 0)
                    elif g == ng - 1:
                        emit_exp(w, 1)
                if i >= LAG:
                    emit_sbuild_seg(*steps[i - LAG])
                if i < n:
                    w, g = steps[i]
                    if g == 2:
                        load_pass1(w + 2)
                    elif g == 6:
                        load_pass2(w + 1)
    nc.compile()
    return nc


def _prep_inputs(x, batch, W1, b1, W2, b2):
    batch = np.asarray(batch).astype(np.int64)
    x = np.asarray(x, dtype=np.float32)

    bnds = np.searchsorted(batch, np.arange(0, N_GRAPHS + 1, WG))
    sizes = np.diff(bnds)
    T = int(max(512, ((int(sizes.max()) + GRP - 1) // GRP) * GRP))
    ng = T // GRP
    cols = T // 128

    xbf = x.astype(BF)
    x8 = x.astype(F8)
    batf = batch.astype(np.float32)

    W1 = np.asarray(W1, np.float32)
    cbf = np.zeros((128, CBW), dtype=BF)
    for k in range(2):
        cbf[:, C_W2 + k] = np.asarray(W2, np.float32)[128 * k:128 * (k + 1), 0].astype(BF)
    cbf[:, C_IOTA:C_IOTA + 128] = np.tile(
        np.arange(128, dtype=np.float32), (128, 1)).astype(BF)

    # W1 DoubleRow: cw1[p, mb, i, m] = W1[p + 128*i, 128*mb + m]
    cw1 = np.zeros((128, 2, 2, 128), dtype=F8)
    for mb in range(2):
        for i in range(2):
            cw1[:, mb, i, :] = \
                W1[128 * i:128 * (i + 1), 128 * mb:128 * (mb + 1)].astype(F8)

    cf = np.zeros((128, 3), dtype=np.float32)
    cf[:, 0] = np.asarray(b1, np.float32)[0:128]
    cf[:, 1] = np.asarray(b1, np.float32)[128:256]
    cf[:, 2] = float(np.asarray(b2, np.float32).reshape(-1)[0])
    zero_bias = bool((np.asarray(b1) == 0).all() and (np.asarray(b2) == 0).all())

    in_maps = []
    for c in range(CORES):
        xn_c = np.zeros((WPC, T, ROW), dtype=BF)
        xt_lin = np.zeros((WPC, 2, 128, T), dtype=F8)
        br_c = np.full((WPC, 128, cols), -1.0, dtype=np.float32)
        for w in range(WPC):
            j = c * WPC + w
            s, e = int(bnds[j]), int(bnds[j + 1])
            sz = e - s
            if sz:
                xn_c[w, :sz, 0:256] = xbf[s:e]
                xn_c[w, :sz, 256] = BF(1.0)
                xt_lin[w, 0, :, :sz] = x8[s:e, 0:128].T
                xt_lin[w, 1, :, :sz] = x8[s:e, 128:256].T
                tmp = np.full(T, -1.0, dtype=np.float32)
                tmp[:sz] = batf[s:e] - (c * 1024 + w * WG)
                br_c[w] = tmp.reshape(cols, 128).T
        # xn swizzle: [w, g*512+t*128+p, d] -> [w, p, (g*4+t)*ROW + d]
        xn_sw = np.ascontiguousarray(
            xn_c.reshape(WPC, ng, 4, 128, ROW).transpose(0, 3, 1, 2, 4)
        ).reshape(WPC, 128, ng * 4 * ROW)
        # xt swizzle: [w, c2, p, g*512+n] -> [w, p, g, c2, n]
        xt_sw = np.ascontiguousarray(
            xt_lin.reshape(WPC, 2, 128, ng, 512).transpose(0, 2, 3, 1, 4))
        in_maps.append(dict(xn=xn_sw, xt=xt_sw, br=br_c, cbf=cbf, cw1=cw1, cf=cf))
    return T, in_maps, zero_bias


_PROGRAM_CACHE = {}


def kernel(x, batch, W1, b1, W2, b2):
    T, in_maps, zb = _prep_inputs(x, batch, W1, b1, W2, b2)
    key = (T, zb)
    if key not in _PROGRAM_CACHE:
        _PROGRAM_CACHE[key] = _build_program(T, zero_bias=zb)
    nc = _PROGRAM_CACHE[key]
    res = run_bass_kernel_spmd(nc, in_maps, list(range(CORES))).results
    return np.concatenate([res[c]["out"] for c in range(CORES)], axis=0)
